# revision 1
# baseline (speedup 1.0000x reference)
"""Trainium2 Bass kernel for Restormer-style transposed (channel) attention.

Math (per batch b):
  qkv = qkv_w @ x                    (1x1 conv, channel GEMM)
  qkv = DWConv3x3(qkv)               (per-channel 3x3, SAME zero pad)
  q,k,v = split(qkv); per head: q,k l2-normalized over spatial
  attn  = softmax((q_n @ k_n^T) * temperature)
  y     = proj_w @ (blockdiag(attn) @ v)

Sharding: 8 cores <-> (batch b = core//2, image-row half = core%2).
Cross-half reductions (channel sumsq + Gram) are tiny: one AllReduce of a
[128,200] f32 stat tile per pair. The final projection is folded with the
attention: y = (proj_w @ blockdiag(attn)) @ v, one 192x192 GEMM applied to v.

Engine layout per core:
  PE : qkv GEMM, q/k transposes, Gram, post matmuls, final GEMM
  DVE: depthwise conv (tensor_scalar + 8x scalar_tensor_tensor per tile),
       softmax small ops, final-GEMM evac
  ACT: PSUM->SBUF evacuations, sumsq (Square+accum), exp
  DMA: x streaming, odd-alignment shifted copy, v scratch, y store
"""

import sys
sys.path.insert(0, '/opt/trn_rl_repo')

import numpy as np
import ml_dtypes
from contextlib import ExitStack

import concourse.bass as bass
import concourse.bacc as bacc
import concourse.tile as tile
import concourse.mybir as mybir
from concourse.bass_utils import run_bass_kernel_spmd

bf16 = mybir.dt.bfloat16
f32 = mybir.dt.float32
Alu = mybir.AluOpType
Act = mybir.ActivationFunctionType

B, C, HEADS, CPH = 4, 192, 4, 48
W = 256
WP = W + 4          # 2 zero cols left, 2 right (keeps 4B alignment)
N_CORES = 8

# o-tile partition map over the 576 qkv channels (q:0-191, k:192-383, v:384-575)
OT = [(0, 128), (128, 128), (256, 128), (384, 128), (512, 64)]
TAPS = [(dy, dx) for dy in (-1, 0, 1) for dx in (-1, 0, 1)]

_BUILT = {}


def build(H=256, CR=16):
    """H: image height (256 real). CR: valid rows per chunk."""
    HALF = H // 2                      # image rows per core
    NCH = HALF // CR                   # chunks
    assert NCH * CR == HALF
    SH_ROWS = HALF + 2                 # shard rows incl. conv halo
    NF = (CR + 2) * WP                 # GEMM window elems per chunk
    NV = CR * W                        # valid conv-out elems per chunk
    NPX = HALF * W                     # valid pixels per core
    PXB = NV // 128                    # 128-px blocks per chunk
    assert PXB % 2 == 0
    VC = 1024                          # final-GEMM v chunk cols
    assert NPX % VC == 0

    nc = bacc.Bacc("TRN2", target_bir_lowering=False, debug=False,
                   num_devices=N_CORES)
    dram = lambda n, s, d, kind: nc.dram_tensor(n, s, d, kind=kind).ap()
    x_d = dram("x", [C, SH_ROWS * WP], bf16, "ExternalInput")
    wt1_d = dram("wt1", [128, 576], bf16, "ExternalInput")
    wt2_d = dram("wt2", [64, 576], bf16, "ExternalInput")
    wdw_d = dram("wdw", [128, 45], f32, "ExternalInput")
    pjt1_d = dram("pjt1", [128, 192], bf16, "ExternalInput")
    pjt2_d = dram("pjt2", [64, 192], bf16, "ExternalInput")
    id_d = dram("ident", [128, 128], bf16, "ExternalInput")
    id64_d = dram("ident64", [128, 64], bf16, "ExternalInput")
    tmp_d = dram("tempb", [48, 4], f32, "ExternalInput")
    y_d = dram("y", [C, NPX], f32, "ExternalOutput")
    dbg_d = dram("dbg", [128, 200], f32, "ExternalOutput")

    with tile.TileContext(nc) as tc, ExitStack() as ctx:
        P = lambda name, bufs, space="SBUF": ctx.enter_context(
            tc.tile_pool(name=name, bufs=bufs, space=space))
        wp = P("wp", 1)
        xp = P("xp", 2)
        qkvp = P("qkvp", 2)
        qsp = P("qsp", 2)
        cop = P("cop", 2)      # per-o-tile tags co0..co4
        sqp = P("sqp", 1)
        qtp = P("qtp", 2)
        stp = P("stp", 1)
        postp = P("postp", 1)
        vcp = P("vcp", 2)
        ysp = P("ysp", 2)
        ps_g = P("ps_g", 2, "PSUM")
        ps_t = P("ps_t", 1, "PSUM")
        ps_gram = P("ps_gram", 1, "PSUM")
        ps_post = P("ps_post", 2, "PSUM")
        drp = P("drp", 1, "DRAM")

        # ---- weights / constants in SBUF
        wt1 = wp.tile([128, 576], bf16, tag="wt1")
        wt2 = wp.tile([64, 576], bf16, tag="wt2")
        wdw = wp.tile([128, 45], f32, tag="wdw")
        pjt1 = wp.tile([128, 192], bf16, tag="pjt1")
        pjt2 = wp.tile([64, 192], bf16, tag="pjt2")
        ident = wp.tile([128, 128], bf16, tag="ident")
        ident64 = wp.tile([128, 64], bf16, tag="ident64")
        tempb = wp.tile([48, 4], f32, tag="tempb")
        for t, d in [(wt1, wt1_d), (wt2, wt2_d), (wdw, wdw_d), (pjt1, pjt1_d),
                     (pjt2, pjt2_d), (ident, id_d), (ident64, id64_d), (tempb, tmp_d)]:
            nc.sync.dma_start(t[:], d[:])

        # persistent accumulators
        ss_acc = stp.tile([128, 3 * NCH], f32, tag="ss")      # sumsq partials
        stats = stp.tile([128, 200], f32, tag="stats")        # pre-AR pack
        stats_rd = stp.tile([128, 200], f32, tag="stats_rd")  # post-AR
        g1 = ps_gram.tile([96, 192], f32, tag="g1")           # Gram c 0..95
        g2 = ps_gram.tile([96, 192], f32, tag="g2")           # Gram c 96..191
        vres = drp.tile([C, NPX], bf16, tag="vres")
        ar_in = drp.tile([128, 200], f32, tag="ar_in")
        ar_out = drp.tile([128, 200], f32, tag="ar_out")

        nc.vector.memset(stats[:], 0.0)

        # ================= main chunk loop =================
        for c in range(NCH):
            xoff = c * CR * WP
            xc1 = xp.tile([128, NF], bf16, tag="xc1")
            xc2 = xp.tile([64, NF], bf16, tag="xc2")
            nc.sync.dma_start(xc1[:], x_d[0:128, xoff:xoff + NF])
            nc.sync.dma_start(xc2[:], x_d[128:192, xoff:xoff + NF])

            cos = []                       # conv-out tiles for q/k o-tiles
            for j, (o0, orows) in enumerate(OT):
                # --- GEMM into psum pieces, evac to SBUF (ACT)
                qk = qkvp.tile([128, NF], bf16, tag="qk")
                p = 0
                while p < NF:
                    pw = min(512, NF - p)
                    ps = ps_g.tile([128, 512], f32, tag="gemm")
                    nc.tensor.matmul(ps[0:orows, 0:pw],
                                     wt1[:, o0:o0 + orows],
                                     xc1[:, p:p + pw], start=True, stop=False)
                    nc.tensor.matmul(ps[0:orows, 0:pw],
                                     wt2[:, o0:o0 + orows],
                                     xc2[:, p:p + pw], start=False, stop=True)
                    nc.scalar.copy(qk[0:orows, p:p + pw], ps[0:orows, 0:pw])
                    p += pw
                # --- odd-aligned sibling (shifted by one element) via DMA
                qs = qsp.tile([128, NF], bf16, tag="qs")
                nc.sync.dma_start(qs[0:orows, 0:NF - 2], qk[0:orows, 1:NF - 1])

                # --- depthwise 3x3: 1 tensor_scalar + 8 scalar_tensor_tensor
                co = cop.tile([128, NV], bf16, tag=f"co{j}")
                co3 = co[0:orows, :].rearrange("p (r c) -> p r c", c=W)
                qk3 = qk[0:orows, :].rearrange("p (r c) -> p r c", c=WP)
                qs3 = qs[0:orows, :].rearrange("p (r c) -> p r c", c=WP)
                for t, (dy, dx) in enumerate(TAPS):
                    r0 = 1 + dy
                    if dx == 0:
                        src = qk3[:, r0:r0 + CR, 2:2 + W]
                    else:
                        cs = 1 + dx  # shifted tile col: qk col (2+dx) - 1
                        src = qs3[:, r0:r0 + CR, cs:cs + W]
                    wsl = wdw[0:orows, 9 * j + t:9 * j + t + 1]
                    if t == 0:
                        nc.vector.tensor_scalar(co3[:], src, wsl, None, Alu.mult)
                    else:
                        nc.vector.scalar_tensor_tensor(
                            co3[:], src, wsl, co3[:], Alu.mult, Alu.add)

                if j >= 3:   # v channels -> DRAM scratch
                    nc.sync.dma_start(
                        vres[o0 - 384:o0 - 384 + orows, c * NV:(c + 1) * NV],
                        co[0:orows, :])
                else:        # q/k channels -> sumsq partial (ACT)
                    cos.append(co)
                    sq = sqp.tile([128, NV], bf16, tag="sq")
                    nc.scalar.activation(
                        sq[0:orows, :], co[0:orows, :], Act.Square,
                        accum_out=ss_acc[0:orows, j * NCH + c:j * NCH + c + 1])

            # --- transposes + Gram over this chunk's q/k
            co0, co1, co2 = cos
            for bb in range(0, PXB, 2):
                qt_ps = ps_t.tile([128, 384], bf16, tag="qt")
                kt_ps = ps_t.tile([128, 384], bf16, tag="kt")
                for s in range(2):
                    blk = (bb + s) * 128
                    ofs = 192 * s
                    nc.tensor.transpose(qt_ps[:, ofs:ofs + 128],
                                        co0[:, blk:blk + 128], ident[:])
                    nc.tensor.transpose(qt_ps[:, ofs + 128:ofs + 192],
                                        co1[0:64, blk:blk + 128],
                                        ident64[0:64, :])
                    nc.tensor.transpose(kt_ps[:, ofs:ofs + 64],
                                        co1[64:128, blk:blk + 128],
                                        ident64[64:128, :])
                    nc.tensor.transpose(kt_ps[:, ofs + 64:ofs + 192],
                                        co2[:, blk:blk + 128], ident[:])
                qt = qtp.tile([128, 384], bf16, tag="qts")
                kt = qtp.tile([128, 384], bf16, tag="kts")
                nc.scalar.copy(qt[:], qt_ps[:])
                nc.scalar.copy(kt[:], kt_ps[:])
                first = (c == 0 and bb == 0)
                last = (c == NCH - 1 and bb == PXB - 2)
                for s in range(2):
                    ofs = 192 * s
                    nc.tensor.matmul(g1[:], qt[:, ofs:ofs + 96],
                                     kt[:, ofs:ofs + 192],
                                     start=(first and s == 0),
                                     stop=(last and s == 1))
                    nc.tensor.matmul(g2[:], qt[:, ofs + 96:ofs + 192],
                                     kt[:, ofs:ofs + 192],
                                     start=(first and s == 0),
                                     stop=(last and s == 1))

        # ================= stats pack + AllReduce =================
        # Gram diagonal blocks -> stats[0:48, 48h:48h+48]
        gsb1 = stp.tile([96, 192], f32, tag="gsb1")
        gsb2 = stp.tile([96, 192], f32, tag="gsb2")
        nc.scalar.copy(gsb1[:], g1[:])
        nc.scalar.copy(gsb2[:], g2[:])
        nc.sync.dma_start(stats[0:48, 0:48], gsb1[0:48, 0:48])
        nc.sync.dma_start(stats[0:48, 48:96], gsb1[48:96, 48:96])
        nc.sync.dma_start(stats[0:48, 96:144], gsb2[0:48, 96:144])
        nc.sync.dma_start(stats[0:48, 144:192], gsb2[48:96, 144:192])
        for j in range(3):
            nc.vector.tensor_reduce(
                stats[0:OT[j][1], 192 + j:193 + j],
                ss_acc[0:OT[j][1], j * NCH:(j + 1) * NCH],
                mybir.AxisListType.X, Alu.add)
        nc.sync.dma_start(ar_in[:], stats[:])
        nc.gpsimd.collective_compute(
            "AllReduce", Alu.add,
            replica_groups=[[0, 1], [2, 3], [4, 5], [6, 7]],
            ins=[ar_in.opt()], outs=[ar_out.opt()])
        nc.sync.dma_start(stats_rd[:], ar_out[:])
        nc.sync.dma_start(dbg_d[:], stats_rd[:])

        # ================= softmax(attn) =================
        # reassemble qss/kss as [48 part, 4 head] via partition-moving DMAs
        qss = postp.tile([48, 4], f32, tag="qss")
        kss = postp.tile([48, 4], f32, tag="kss")
        mv = [
            (qss, 0, 0, 48, 192, 0), (qss, 1, 0, 48, 192, 48),
            (qss, 2, 0, 32, 192, 96), (qss, 2, 32, 16, 193, 0),
            (qss, 3, 0, 48, 193, 16),
            (kss, 0, 0, 48, 193, 64), (kss, 1, 0, 16, 193, 112),
            (kss, 1, 16, 32, 194, 0), (kss, 2, 0, 48, 194, 32),
            (kss, 3, 0, 48, 194, 80),
        ]
        for dst, h, dp, n, col, sp in mv:
            nc.sync.dma_start(dst[dp:dp + n, h:h + 1],
                              stats_rd[sp:sp + n, col:col + 1])
        rq = postp.tile([48, 4], f32, tag="rq")
        rk = postp.tile([48, 4], f32, tag="rk")
        nc.scalar.sqrt(rq[:], qss[:])
        nc.scalar.sqrt(rk[:], kss[:])
        nc.vector.reciprocal(rq[:], rq[:])
        nc.vector.reciprocal(rk[:], rk[:])
        nc.vector.tensor_tensor(rq[:], rq[:], tempb[:], Alu.mult)
        # rk as a [1,192] row (h-major): dst free idx 48h+d
        rk_row = postp.tile([1, 192], f32, tag="rkrow")
        for h in range(4):
            nc.sync.dma_start(rk_row[0:1, 48 * h:48 * h + 48], rk[:, h:h + 1])
        rk_row_b = postp.tile([1, 192], bf16, tag="rkrowb")
        nc.vector.tensor_copy(rk_row_b[:], rk_row[:])
        ones_f = postp.tile([1, 48], bf16, tag="onesf")
        nc.vector.memset(ones_f[:], 1.0)
        rk_bc = ps_post.tile([48, 192], f32, tag="post")
        nc.tensor.matmul(rk_bc[:], ones_f[:], rk_row_b[:],
                         start=True, stop=True)
        logits = postp.tile([48, 192], f32, tag="logits")
        for h in range(4):
            sl = slice(48 * h, 48 * h + 48)
            nc.vector.tensor_scalar(logits[:, sl], stats_rd[0:48, sl],
                                    rq[:, h:h + 1], None, Alu.mult)
        nc.vector.tensor_tensor(logits[:], logits[:], rk_bc[:], Alu.mult)
        l3 = logits[:].rearrange("p (h d) -> p h d", h=4)
        rmax = postp.tile([48, 4], f32, tag="rmax")
        nc.vector.tensor_reduce(rmax[:], l3, mybir.AxisListType.X, Alu.max)
        for h in range(4):
            sl = slice(48 * h, 48 * h + 48)
            nc.vector.tensor_scalar(logits[:, sl], logits[:, sl],
                                    rmax[:, h:h + 1], None, Alu.subtract)
        nc.scalar.activation(logits[:], logits[:], Act.Exp)
        rsum = postp.tile([48, 4], f32, tag="rsum")
        nc.vector.tensor_reduce(rsum[:], l3, mybir.AxisListType.X, Alu.add)
        nc.vector.reciprocal(rsum[:], rsum[:])
        attn = postp.tile([48, 192], bf16, tag="attn")
        for h in range(4):
            sl = slice(48 * h, 48 * h + 48)
            nc.vector.tensor_scalar(attn[:, sl], logits[:, sl],
                                    rsum[:, h:h + 1], None, Alu.mult)

        # ================= M_bT = BD^T @ projT =================
        # lhsT slot (48h+a, 48h+b) must hold attn_h[a, b]: copy attn directly
        bd1 = postp.tile([128, 192], bf16, tag="bd1")
        bd2 = postp.tile([64, 192], bf16, tag="bd2")
        nc.vector.memset(bd1[:], 0.0)
        nc.vector.memset(bd2[:], 0.0)
        nc.sync.dma_start(bd1[0:48, 0:48], attn[:, 0:48])
        nc.sync.dma_start(bd1[48:96, 48:96], attn[:, 48:96])
        nc.sync.dma_start(bd1[96:128, 96:144], attn[0:32, 96:144])
        nc.sync.dma_start(bd2[0:16, 96:144], attn[32:48, 96:144])
        nc.sync.dma_start(bd2[16:64, 144:192], attn[:, 144:192])
        mbt_ps1 = ps_post.tile([128, 192], f32, tag="post")
        nc.tensor.matmul(mbt_ps1[:], bd1[:, 0:128], pjt1[:], start=True, stop=False)
        nc.tensor.matmul(mbt_ps1[:], bd2[:, 0:128], pjt2[:], start=False, stop=True)
        mbt1 = postp.tile([128, 192], bf16, tag="mbt1")
        nc.scalar.copy(mbt1[:], mbt_ps1[:])
        mbt_ps2 = ps_post.tile([64, 192], f32, tag="post")
        nc.tensor.matmul(mbt_ps2[:], bd1[:, 128:192], pjt1[:], start=True, stop=False)
        nc.tensor.matmul(mbt_ps2[:], bd2[:, 128:192], pjt2[:], start=False, stop=True)
        mbt2 = postp.tile([64, 192], bf16, tag="mbt2")
        nc.scalar.copy(mbt2[:], mbt_ps2[:])

        # ================= y = M_b @ v =================
        for vc in range(NPX // VC):
            v1 = vcp.tile([128, VC], bf16, tag="v1")
            v2 = vcp.tile([64, VC], bf16, tag="v2")
            nc.sync.dma_start(v1[:], vres[0:128, vc * VC:(vc + 1) * VC])
            nc.sync.dma_start(v2[:], vres[128:192, vc * VC:(vc + 1) * VC])
            y1 = ysp.tile([128, VC], f32, tag="y1")
            y2 = ysp.tile([64, VC], f32, tag="y2")
            for p in range(0, VC, 512):
                ps = ps_g.tile([128, 512], f32, tag="gemm")
                nc.tensor.matmul(ps[:], mbt1[:, 0:128], v1[:, p:p + 512],
                                 start=True, stop=False)
                nc.tensor.matmul(ps[:], mbt2[:, 0:128], v2[:, p:p + 512],
                                 start=False, stop=True)
                nc.vector.tensor_copy(y1[:, p:p + 512], ps[:])
                ps2 = ps_g.tile([128, 512], f32, tag="gemm")
                nc.tensor.matmul(ps2[0:64, :], mbt1[:, 128:192], v1[:, p:p + 512],
                                 start=True, stop=False)
                nc.tensor.matmul(ps2[0:64, :], mbt2[:, 128:192], v2[:, p:p + 512],
                                 start=False, stop=True)
                nc.vector.tensor_copy(y2[:, p:p + 512], ps2[0:64, :])
            nc.sync.dma_start(y_d[0:128, vc * VC:(vc + 1) * VC], y1[:])
            nc.sync.dma_start(y_d[128:192, vc * VC:(vc + 1) * VC], y2[:])

    nc.compile()
    return nc


def _host_pack(x, qkv_w, dw_w, proj_w, temperature, H):
    HALF = H // 2
    SH_ROWS = HALF + 2
    bfa = lambda a: np.ascontiguousarray(a.astype(ml_dtypes.bfloat16))
    wT = qkv_w.T.astype(np.float32)                     # [192, 576]
    dw9 = dw_w.reshape(576, 9).astype(np.float32)
    wdw = np.zeros((128, 45), np.float32)
    for j, (o0, orows) in enumerate(OT):
        wdw[0:orows, 9 * j:9 * j + 9] = dw9[o0:o0 + orows]
    pjT = proj_w.T.astype(np.float32)
    shared = {
        "wt1": bfa(wT[0:128]), "wt2": bfa(wT[128:192]), "wdw": wdw,
        "pjt1": bfa(pjT[0:128]), "pjt2": bfa(pjT[128:192]),
        "ident": bfa(np.eye(128, dtype=np.float32)),
        "ident64": bfa(np.vstack([np.eye(64, dtype=np.float32)] * 2)),
        "tempb": np.ascontiguousarray(np.broadcast_to(
            np.asarray(temperature, np.float32).reshape(1, HEADS),
            (48, HEADS)).astype(np.float32)),
    }
    in_maps = []
    for core in range(N_CORES):
        b, h = core // 2, core % 2
        xs = np.zeros((C, SH_ROWS, WP), np.float32)
        r0 = h * HALF - 1
        lo, hi = max(r0, 0), min(r0 + SH_ROWS, H)
        xs[:, lo - r0:hi - r0, 2:2 + W] = x[b][:, lo:hi, :]
        in_maps.append({**shared, "x": bfa(xs.reshape(C, SH_ROWS * WP))})
    return in_maps


def kernel(x, qkv_w, dw_w, proj_w, temperature, num_heads):
    x = np.asarray(x, np.float32)
    H = x.shape[2]
    assert int(num_heads) == HEADS and x.shape == (B, C, H, W)
    key = (H,)
    if key not in _BUILT:
        _BUILT[key] = build(H=H, CR=16 if (H // 2) % 16 == 0 else H // 2)
    nc = _BUILT[key]
    in_maps = _host_pack(x, np.asarray(qkv_w, np.float32),
                         np.asarray(dw_w, np.float32),
                         np.asarray(proj_w, np.float32),
                         np.asarray(temperature, np.float32).reshape(-1), H)
    res = run_bass_kernel_spmd(nc, in_maps, list(range(N_CORES)))
    HALF = H // 2
    out = np.empty((B, C, H, W), np.float32)
    for core in range(N_CORES):
        b, h = core // 2, core % 2
        out[b, :, h * HALF:(h + 1) * HALF, :] = \
            res.results[core]["y"].reshape(C, HALF, W)
    return out


def build_empty(H=256):
    """Same external IO as build(), trivial body — for launch-overhead calibration."""
    HALF = H // 2
    SH_ROWS = HALF + 2
    NPX = HALF * W
    nc = bacc.Bacc("TRN2", target_bir_lowering=False, debug=False,
                   num_devices=N_CORES)
    dram = lambda n, s, d, kind: nc.dram_tensor(n, s, d, kind=kind).ap()
    x_d = dram("x", [C, SH_ROWS * WP], bf16, "ExternalInput")
    dram("wt1", [128, 576], bf16, "ExternalInput")
    dram("wt2", [64, 576], bf16, "ExternalInput")
    dram("wdw", [128, 45], f32, "ExternalInput")
    dram("pjt1", [128, 192], bf16, "ExternalInput")
    dram("pjt2", [64, 192], bf16, "ExternalInput")
    dram("ident", [128, 128], bf16, "ExternalInput")
    dram("ident64", [128, 64], bf16, "ExternalInput")
    dram("tempb", [48, 4], f32, "ExternalInput")
    dram("y", [C, NPX], f32, "ExternalOutput")
    dbg_d = dram("dbg", [128, 200], f32, "ExternalOutput")
    with tile.TileContext(nc) as tc, ExitStack() as ctx:
        sb = ctx.enter_context(tc.tile_pool(name="sb", bufs=1))
        t = sb.tile([128, 200], bf16)
        nc.sync.dma_start(t[:, 0:169], x_d[0:128, 0:169])
        t2 = sb.tile([128, 200], f32)
        nc.vector.tensor_copy(t2[:, 0:169], t[:, 0:169])
        nc.sync.dma_start(dbg_d[:, 0:169], t2[:, 0:169])
    nc.compile()
    return nc



# revision 4
# speedup vs baseline: 3.9859x; 3.9859x over previous
"""Trainium2 Bass kernel for Restormer-style transposed (channel) attention.

Math (per batch b):
  qkv = qkv_w @ x                    (1x1 conv, channel GEMM)
  qkv = DWConv3x3(qkv)               (per-channel 3x3, SAME zero pad)
  q,k,v = split(qkv); per head: q,k l2-normalized over spatial
  attn  = softmax((q_n @ k_n^T) * temperature)
  y     = proj_w @ (blockdiag(attn) @ v)

Sharding: 8 cores <-> (batch b = core//2, image-row half = core%2).
Cross-half reductions (channel sumsq + Gram) are tiny: one AllReduce of a
[128,200] f32 stat tile per pair. The final projection is folded with the
attention: y = (proj_w @ blockdiag(attn)) @ v, one 192x192 GEMM applied to v.

v2 changes vs v1:
  - q/k pixel-major tiles come from XBAR DMA transposes (4 dma_start
    (transpose=True) per chunk) instead of ~224 PE transpose+evac
    instructions per chunk; Gram shrinks to head-pair [96,96] matmuls.
  - qkv GEMM evacuates PSUM in 1024-wide pieces (5 copies/o-tile vs 10).
  - dbg output dropped, y stored bf16 (halves result fetch).
  - kernel() runs through a cached jax.jit executable (no re-trace /
    re-lowering / bir_verify per call).

Engine layout per core:
  PE : qkv GEMM, Gram (head-pair), post matmuls, final y GEMM
  DVE: depthwise conv (tensor_scalar + 8x scalar_tensor_tensor per tile),
       softmax small ops
  ACT: PSUM->SBUF evacuations, sumsq (Square+accum), exp
  DMA: x streaming, odd-alignment shifted copy, q/k XBAR transposes,
       v scratch, y store
"""

import sys
sys.path.insert(0, '/opt/trn_rl_repo')

import numpy as np
import ml_dtypes
from contextlib import ExitStack

import concourse.bass as bass
import concourse.bacc as bacc
import concourse.tile as tile
import concourse.mybir as mybir

bf16 = mybir.dt.bfloat16
f32 = mybir.dt.float32
Alu = mybir.AluOpType
Act = mybir.ActivationFunctionType

B, C, HEADS, CPH = 4, 192, 4, 48
W = 256
WP = W + 4          # 2 zero cols left, 2 right (keeps 4B alignment)
N_CORES = 8

# o-tile partition map over the 576 qkv channels (q:0-191, k:192-383, v:384-575)
OT = [(0, 128), (128, 128), (256, 128), (384, 128), (512, 64)]
TAPS = [(dy, dx) for dy in (-1, 0, 1) for dx in (-1, 0, 1)]

_BUILT = {}


def build(H=256, CR=16):
    """H: image height (256 real). CR: valid rows per chunk."""
    HALF = H // 2                      # image rows per core
    NCH = HALF // CR                   # chunks
    assert NCH * CR == HALF
    SH_ROWS = HALF + 2                 # shard rows incl. conv halo
    NF = (CR + 2) * WP                 # GEMM window elems per chunk
    NV = CR * W                        # valid conv-out elems per chunk
    NPX = HALF * W                     # valid pixels per core
    PXB = NV // 128                    # 128-px blocks per chunk
    VC = 1024                          # final-GEMM v chunk cols
    assert NPX % VC == 0

    nc = bacc.Bacc("TRN2", target_bir_lowering=False, debug=False,
                   num_devices=N_CORES)
    dram = lambda n, s, d, kind: nc.dram_tensor(n, s, d, kind=kind).ap()
    x_d = dram("x", [C, SH_ROWS * WP], bf16, "ExternalInput")
    wt1_d = dram("wt1", [128, 576], bf16, "ExternalInput")
    wt2_d = dram("wt2", [64, 576], bf16, "ExternalInput")
    wdw_d = dram("wdw", [128, 45], f32, "ExternalInput")
    pjt1_d = dram("pjt1", [128, 192], bf16, "ExternalInput")
    pjt2_d = dram("pjt2", [64, 192], bf16, "ExternalInput")
    tmp_d = dram("tempb", [48, 4], f32, "ExternalInput")
    y_d = dram("y", [C, NPX], bf16, "ExternalOutput")

    with tile.TileContext(nc) as tc, ExitStack() as ctx:
        P = lambda name, bufs, space="SBUF": ctx.enter_context(
            tc.tile_pool(name=name, bufs=bufs, space=space))
        wp = P("wp", 1)
        xp = P("xp", 2)
        qkvp = P("qkvp", 2)
        qsp = P("qsp", 2)
        cop = P("cop", 2)      # q/k conv-out tags co0..co2
        covp = P("covp", 1)    # v conv-out tags co3..co4 (DMA'd out fast)
        sqp = P("sqp", 1)
        qtp = P("qtp", 1)
        stp = P("stp", 1)
        postp = P("postp", 1)
        vcp = P("vcp", 2)
        ysp = P("ysp", 2)
        ps_g = P("ps_g", 2, "PSUM")
        ps_gram = P("ps_gram", 1, "PSUM")
        ps_post = P("ps_post", 2, "PSUM")
        drp = P("drp", 1, "DRAM")

        # ---- weights / constants in SBUF
        wt1 = wp.tile([128, 576], bf16, tag="wt1")
        wt2 = wp.tile([64, 576], bf16, tag="wt2")
        wdw = wp.tile([128, 45], f32, tag="wdw")
        pjt1 = wp.tile([128, 192], bf16, tag="pjt1")
        pjt2 = wp.tile([64, 192], bf16, tag="pjt2")
        tempb = wp.tile([48, 4], f32, tag="tempb")
        for t, d in [(wt1, wt1_d), (wt2, wt2_d), (wdw, wdw_d),
                     (pjt1, pjt1_d), (pjt2, pjt2_d), (tempb, tmp_d)]:
            nc.sync.dma_start(t[:], d[:])

        # persistent accumulators
        ss_acc = stp.tile([128, 3 * NCH], f32, tag="ss")      # sumsq partials
        stats = stp.tile([128, 200], f32, tag="stats")        # pre-AR pack
        stats_rd = stp.tile([128, 200], f32, tag="stats_rd")  # post-AR
        g01 = ps_gram.tile([96, 96], f32, tag="g01")          # Gram heads 0,1
        g23 = ps_gram.tile([96, 96], f32, tag="g23")          # Gram heads 2,3
        vres = drp.tile([C, NPX], bf16, tag="vres")
        ar_in = drp.tile([128, 200], f32, tag="ar_in")
        ar_out = drp.tile([128, 200], f32, tag="ar_out")

        nc.vector.memset(stats[:], 0.0)

        # ================= main chunk loop =================
        for c in range(NCH):
            xoff = c * CR * WP
            xc1 = xp.tile([128, NF], bf16, tag="xc1")
            xc2 = xp.tile([64, NF], bf16, tag="xc2")
            nc.sync.dma_start(xc1[:], x_d[0:128, xoff:xoff + NF])
            nc.sync.dma_start(xc2[:], x_d[128:192, xoff:xoff + NF])

            cos = []                       # conv-out tiles for q/k o-tiles
            for j, (o0, orows) in enumerate(OT):
                # --- GEMM into 1024-wide psum pieces, evac to SBUF (ACT)
                qk = qkvp.tile([128, NF], bf16, tag="qk")
                p = 0
                while p < NF:
                    pw = min(1024, NF - p)
                    ps = ps_g.tile([128, 1024], f32, tag="gemm")
                    q0 = 0
                    while q0 < pw:
                        sw = min(512, pw - q0)
                        nc.tensor.matmul(ps[0:orows, q0:q0 + sw],
                                         wt1[:, o0:o0 + orows],
                                         xc1[:, p + q0:p + q0 + sw],
                                         start=True, stop=False)
                        nc.tensor.matmul(ps[0:orows, q0:q0 + sw],
                                         wt2[:, o0:o0 + orows],
                                         xc2[:, p + q0:p + q0 + sw],
                                         start=False, stop=True)
                        q0 += sw
                    nc.scalar.copy(qk[0:orows, p:p + pw], ps[0:orows, 0:pw])
                    p += pw
                # --- odd-aligned sibling (shifted by one element) via DMA
                qs = qsp.tile([128, NF], bf16, tag="qs")
                nc.sync.dma_start(qs[0:orows, 0:NF - 2], qk[0:orows, 1:NF - 1])

                # --- depthwise 3x3: 1 tensor_scalar + 8 scalar_tensor_tensor
                pool_j = cop if j < 3 else covp
                co = pool_j.tile([128, NV], bf16, tag=f"co{j}")
                co3 = co[0:orows, :].rearrange("p (r c) -> p r c", c=W)
                qk3 = qk[0:orows, :].rearrange("p (r c) -> p r c", c=WP)
                qs3 = qs[0:orows, :].rearrange("p (r c) -> p r c", c=WP)
                for t, (dy, dx) in enumerate(TAPS):
                    r0 = 1 + dy
                    if dx == 0:
                        src = qk3[:, r0:r0 + CR, 2:2 + W]
                    else:
                        cs = 1 + dx  # shifted tile col: qk col (2+dx) - 1
                        src = qs3[:, r0:r0 + CR, cs:cs + W]
                    wsl = wdw[0:orows, 9 * j + t:9 * j + t + 1]
                    if t == 0:
                        nc.vector.tensor_scalar(co3[:], src, wsl, None, Alu.mult)
                    else:
                        nc.vector.scalar_tensor_tensor(
                            co3[:], src, wsl, co3[:], Alu.mult, Alu.add)

                if j >= 3:   # v channels -> DRAM scratch
                    nc.sync.dma_start(
                        vres[o0 - 384:o0 - 384 + orows, c * NV:(c + 1) * NV],
                        co[0:orows, :])
                else:        # q/k channels -> sumsq partial (ACT)
                    cos.append(co)
                    sq = sqp.tile([128, NV], bf16, tag="sq")
                    nc.scalar.activation(
                        sq[0:orows, :], co[0:orows, :], Act.Square,
                        accum_out=ss_acc[0:orows, j * NCH + c:j * NCH + c + 1])

            # --- XBAR transposes: q/k -> pixel-major [128px, blk, 192ch]
            co0, co1, co2 = cos
            qt = qtp.tile([128, PXB * 192], bf16, tag="qt")
            kt = qtp.tile([128, PXB * 192], bf16, tag="kt")
            qt3 = qt[:].rearrange("p (b c) -> p b c", c=192)
            kt3 = kt[:].rearrange("p (b c) -> p b c", c=192)
            nc.sync.dma_start(qt3[:, :, 0:128], co0[:, :], transpose=True)
            nc.sync.dma_start(qt3[:, :, 128:192], co1[0:64, :], transpose=True)
            nc.sync.dma_start(kt3[:, :, 0:64], co1[64:128, :], transpose=True)
            nc.sync.dma_start(kt3[:, :, 64:192], co2[:, :], transpose=True)

            # --- Gram accumulation, head pairs (0,1) and (2,3)
            for blk in range(PXB):
                first = (c == 0 and blk == 0)
                last = (c == NCH - 1 and blk == PXB - 1)
                nc.tensor.matmul(g01[:], qt3[:, blk, 0:96],
                                 kt3[:, blk, 0:96], start=first, stop=last)
                nc.tensor.matmul(g23[:], qt3[:, blk, 96:192],
                                 kt3[:, blk, 96:192], start=first, stop=last)

        # ================= stats pack + AllReduce =================
        # Gram diagonal blocks -> stats[0:48, 48h:48h+48]
        gsb1 = stp.tile([96, 96], f32, tag="gsb1")
        gsb2 = stp.tile([96, 96], f32, tag="gsb2")
        nc.scalar.copy(gsb1[:], g01[:])
        nc.scalar.copy(gsb2[:], g23[:])
        nc.sync.dma_start(stats[0:48, 0:48], gsb1[0:48, 0:48])
        nc.sync.dma_start(stats[0:48, 48:96], gsb1[48:96, 48:96])
        nc.sync.dma_start(stats[0:48, 96:144], gsb2[0:48, 0:48])
        nc.sync.dma_start(stats[0:48, 144:192], gsb2[48:96, 48:96])
        for j in range(3):
            nc.vector.tensor_reduce(
                stats[0:OT[j][1], 192 + j:193 + j],
                ss_acc[0:OT[j][1], j * NCH:(j + 1) * NCH],
                mybir.AxisListType.X, Alu.add)
        nc.sync.dma_start(ar_in[:], stats[:])
        nc.gpsimd.collective_compute(
            "AllReduce", Alu.add,
            replica_groups=[[0, 1], [2, 3], [4, 5], [6, 7]],
            ins=[ar_in.opt()], outs=[ar_out.opt()])
        nc.sync.dma_start(stats_rd[:], ar_out[:])

        # ================= softmax(attn) =================
        # reassemble qss/kss as [48 part, 4 head] via partition-moving DMAs
        qss = postp.tile([48, 4], f32, tag="qss")
        kss = postp.tile([48, 4], f32, tag="kss")
        mv = [
            (qss, 0, 0, 48, 192, 0), (qss, 1, 0, 48, 192, 48),
            (qss, 2, 0, 32, 192, 96), (qss, 2, 32, 16, 193, 0),
            (qss, 3, 0, 48, 193, 16),
            (kss, 0, 0, 48, 193, 64), (kss, 1, 0, 16, 193, 112),
            (kss, 1, 16, 32, 194, 0), (kss, 2, 0, 48, 194, 32),
            (kss, 3, 0, 48, 194, 80),
        ]
        for dst, h, dp, n, col, sp in mv:
            nc.sync.dma_start(dst[dp:dp + n, h:h + 1],
                              stats_rd[sp:sp + n, col:col + 1])
        rq = postp.tile([48, 4], f32, tag="rq")
        rk = postp.tile([48, 4], f32, tag="rk")
        nc.scalar.sqrt(rq[:], qss[:])
        nc.scalar.sqrt(rk[:], kss[:])
        nc.vector.reciprocal(rq[:], rq[:])
        nc.vector.reciprocal(rk[:], rk[:])
        nc.vector.tensor_tensor(rq[:], rq[:], tempb[:], Alu.mult)
        # rk as a [1,192] row (h-major): dst free idx 48h+d
        rk_row = postp.tile([1, 192], f32, tag="rkrow")
        for h in range(4):
            nc.sync.dma_start(rk_row[0:1, 48 * h:48 * h + 48], rk[:, h:h + 1])
        rk_row_b = postp.tile([1, 192], bf16, tag="rkrowb")
        nc.vector.tensor_copy(rk_row_b[:], rk_row[:])
        ones_f = postp.tile([1, 48], bf16, tag="onesf")
        nc.vector.memset(ones_f[:], 1.0)
        rk_bc = ps_post.tile([48, 192], f32, tag="post")
        nc.tensor.matmul(rk_bc[:], ones_f[:], rk_row_b[:],
                         start=True, stop=True)
        logits = postp.tile([48, 192], f32, tag="logits")
        for h in range(4):
            sl = slice(48 * h, 48 * h + 48)
            nc.vector.tensor_scalar(logits[:, sl], stats_rd[0:48, sl],
                                    rq[:, h:h + 1], None, Alu.mult)
        nc.vector.tensor_tensor(logits[:], logits[:], rk_bc[:], Alu.mult)
        l3 = logits[:].rearrange("p (h d) -> p h d", h=4)
        rmax = postp.tile([48, 4], f32, tag="rmax")
        nc.vector.tensor_reduce(rmax[:], l3, mybir.AxisListType.X, Alu.max)
        for h in range(4):
            sl = slice(48 * h, 48 * h + 48)
            nc.vector.tensor_scalar(logits[:, sl], logits[:, sl],
                                    rmax[:, h:h + 1], None, Alu.subtract)
        nc.scalar.activation(logits[:], logits[:], Act.Exp)
        rsum = postp.tile([48, 4], f32, tag="rsum")
        nc.vector.tensor_reduce(rsum[:], l3, mybir.AxisListType.X, Alu.add)
        nc.vector.reciprocal(rsum[:], rsum[:])
        attn = postp.tile([48, 192], bf16, tag="attn")
        for h in range(4):
            sl = slice(48 * h, 48 * h + 48)
            nc.vector.tensor_scalar(attn[:, sl], logits[:, sl],
                                    rsum[:, h:h + 1], None, Alu.mult)

        # ================= M_bT = BD^T @ projT =================
        # lhsT slot (48h+a, 48h+b) must hold attn_h[a, b]: copy attn directly
        bd1 = postp.tile([128, 192], bf16, tag="bd1")
        bd2 = postp.tile([64, 192], bf16, tag="bd2")
        nc.vector.memset(bd1[:], 0.0)
        nc.vector.memset(bd2[:], 0.0)
        nc.sync.dma_start(bd1[0:48, 0:48], attn[:, 0:48])
        nc.sync.dma_start(bd1[48:96, 48:96], attn[:, 48:96])
        nc.sync.dma_start(bd1[96:128, 96:144], attn[0:32, 96:144])
        nc.sync.dma_start(bd2[0:16, 96:144], attn[32:48, 96:144])
        nc.sync.dma_start(bd2[16:64, 144:192], attn[:, 144:192])
        mbt_ps1 = ps_post.tile([128, 192], f32, tag="post")
        nc.tensor.matmul(mbt_ps1[:], bd1[:, 0:128], pjt1[:], start=True, stop=False)
        nc.tensor.matmul(mbt_ps1[:], bd2[:, 0:128], pjt2[:], start=False, stop=True)
        mbt1 = postp.tile([128, 192], bf16, tag="mbt1")
        nc.scalar.copy(mbt1[:], mbt_ps1[:])
        mbt_ps2 = ps_post.tile([64, 192], f32, tag="post")
        nc.tensor.matmul(mbt_ps2[:], bd1[:, 128:192], pjt1[:], start=True, stop=False)
        nc.tensor.matmul(mbt_ps2[:], bd2[:, 128:192], pjt2[:], start=False, stop=True)
        mbt2 = postp.tile([64, 192], bf16, tag="mbt2")
        nc.scalar.copy(mbt2[:], mbt_ps2[:])

        # ================= y = M_b @ v =================
        for vc in range(NPX // VC):
            v1 = vcp.tile([128, VC], bf16, tag="v1")
            v2 = vcp.tile([64, VC], bf16, tag="v2")
            nc.sync.dma_start(v1[:], vres[0:128, vc * VC:(vc + 1) * VC])
            nc.sync.dma_start(v2[:], vres[128:192, vc * VC:(vc + 1) * VC])
            y1 = ysp.tile([128, VC], bf16, tag="y1")
            y2 = ysp.tile([64, VC], bf16, tag="y2")
            ps = ps_g.tile([128, 1024], f32, tag="gemm")
            for q0 in (0, 512):
                nc.tensor.matmul(ps[:, q0:q0 + 512], mbt1[:, 0:128],
                                 v1[:, q0:q0 + 512], start=True, stop=False)
                nc.tensor.matmul(ps[:, q0:q0 + 512], mbt2[:, 0:128],
                                 v2[:, q0:q0 + 512], start=False, stop=True)
            nc.scalar.copy(y1[:], ps[:])
            ps2 = ps_g.tile([128, 1024], f32, tag="gemm")
            for q0 in (0, 512):
                nc.tensor.matmul(ps2[0:64, q0:q0 + 512], mbt1[:, 128:192],
                                 v1[:, q0:q0 + 512], start=True, stop=False)
                nc.tensor.matmul(ps2[0:64, q0:q0 + 512], mbt2[:, 128:192],
                                 v2[:, q0:q0 + 512], start=False, stop=True)
            nc.scalar.copy(y2[:], ps2[0:64, :])
            nc.sync.dma_start(y_d[0:128, vc * VC:(vc + 1) * VC], y1[:])
            nc.sync.dma_start(y_d[128:192, vc * VC:(vc + 1) * VC], y2[:])

    nc.compile()
    return nc


def _make_runner(nc, n_cores=N_CORES):
    """Build a cached jax.jit executable for nc once; return a run(in_maps)
    callable. Mirrors bass2jax.run_bass_via_pjrt's multi-core branch, but
    hoists tracing/lowering out of the per-call path."""
    import jax
    from jax.experimental.shard_map import shard_map
    from jax.sharding import Mesh, PartitionSpec
    from concourse import bass2jax

    bass2jax.install_neuronx_cc_hook()
    partition_name = (nc.partition_id_tensor.name
                      if nc.partition_id_tensor else None)
    dbg_name = nc.dbg_addr.name if nc.dbg_addr is not None else None

    in_names, out_names, out_avals, zero_outs = [], [], [], []
    for alloc in nc.m.functions[0].allocations:
        if not isinstance(alloc, mybir.MemoryLocationSet):
            continue
        name = alloc.memorylocations[0].name
        if alloc.kind == "ExternalInput":
            if name != partition_name:
                in_names.append(name)
        elif alloc.kind == "ExternalOutput":
            shape = tuple(alloc.tensor_shape)
            dtype = mybir.dt.np(alloc.dtype)
            out_names.append(name)
            out_avals.append(jax.core.ShapedArray(shape, dtype))
            zero_outs.append(np.zeros(shape, dtype))
    n_params = len(in_names)
    n_outs = len(out_names)
    all_names = list(in_names) + list(out_names)
    if partition_name is not None:
        all_names.append(partition_name)
    donate = tuple(range(n_params, n_params + n_outs))

    def _body(*args):
        operands = list(args)
        if partition_name is not None:
            operands.append(bass2jax.partition_id_tensor())
        outs = bass2jax._bass_exec_p.bind(
            *operands, out_avals=tuple(out_avals), in_names=tuple(all_names),
            out_names=tuple(out_names), lowering_input_output_aliases=(),
            sim_require_finite=True, sim_require_nnan=True, nc=nc)
        return tuple(outs)

    devices = jax.devices()[:n_cores]
    assert len(devices) == n_cores
    mesh = Mesh(np.asarray(devices), ("core",))
    in_specs = (PartitionSpec("core"),) * (n_params + n_outs)
    out_specs = (PartitionSpec("core"),) * n_outs
    sharded = jax.jit(
        shard_map(_body, mesh=mesh, in_specs=in_specs, out_specs=out_specs,
                  check_rep=False),
        donate_argnums=donate, keep_unused=True)

    def run(in_maps):
        per_core = []
        for m in in_maps:
            if dbg_name is not None:
                m = {**m, dbg_name: np.zeros((1, 2), np.uint32)}
            per_core.append([np.asarray(m[name]) for name in in_names])
        concat_in = [
            np.concatenate([per_core[cc][i] for cc in range(n_cores)], axis=0)
            for i in range(n_params)]
        concat_zeros = [
            np.zeros((n_cores * z.shape[0], *z.shape[1:]), z.dtype)
            for z in zero_outs]
        out_arrs = sharded(*concat_in, *concat_zeros)
        np_outs = [np.asarray(a) for a in out_arrs]
        return [
            {name: np_outs[i].reshape(n_cores, *out_avals[i].shape)[cc]
             for i, name in enumerate(out_names)}
            for cc in range(n_cores)]
    return run


def _host_pack(x, qkv_w, dw_w, proj_w, temperature, H):
    HALF = H // 2
    SH_ROWS = HALF + 2
    bfa = lambda a: np.ascontiguousarray(a.astype(ml_dtypes.bfloat16))
    wT = qkv_w.T.astype(np.float32)                     # [192, 576]
    dw9 = dw_w.reshape(576, 9).astype(np.float32)
    wdw = np.zeros((128, 45), np.float32)
    for j, (o0, orows) in enumerate(OT):
        wdw[0:orows, 9 * j:9 * j + 9] = dw9[o0:o0 + orows]
    pjT = proj_w.T.astype(np.float32)
    shared = {
        "wt1": bfa(wT[0:128]), "wt2": bfa(wT[128:192]), "wdw": wdw,
        "pjt1": bfa(pjT[0:128]), "pjt2": bfa(pjT[128:192]),
        "tempb": np.ascontiguousarray(np.broadcast_to(
            np.asarray(temperature, np.float32).reshape(1, HEADS),
            (48, HEADS)).astype(np.float32)),
    }
    in_maps = []
    for core in range(N_CORES):
        b, h = core // 2, core % 2
        xs = np.zeros((C, SH_ROWS, WP), np.float32)
        r0 = h * HALF - 1
        lo, hi = max(r0, 0), min(r0 + SH_ROWS, H)
        xs[:, lo - r0:hi - r0, 2:2 + W] = x[b][:, lo:hi, :]
        in_maps.append({**shared, "x": bfa(xs.reshape(C, SH_ROWS * WP))})
    return in_maps


def _get_built(H):
    key = (H,)
    if key not in _BUILT:
        nc = build(H=H, CR=16 if (H // 2) % 16 == 0 else H // 2)
        _BUILT[key] = (nc, _make_runner(nc))
    return _BUILT[key]


def kernel(x, qkv_w, dw_w, proj_w, temperature, num_heads):
    x = np.asarray(x, np.float32)
    H = x.shape[2]
    assert int(num_heads) == HEADS and x.shape == (B, C, H, W)
    nc, run = _get_built(H)
    in_maps = _host_pack(x, np.asarray(qkv_w, np.float32),
                         np.asarray(dw_w, np.float32),
                         np.asarray(proj_w, np.float32),
                         np.asarray(temperature, np.float32).reshape(-1), H)
    res = run(in_maps)
    HALF = H // 2
    out = np.empty((B, C, H, W), np.float32)
    for core in range(N_CORES):
        b, h = core // 2, core % 2
        out[b, :, h * HALF:(h + 1) * HALF, :] = \
            res[core]["y"].astype(np.float32).reshape(C, HALF, W)
    return out


_EMPTY = {}


def build_empty(H=256):
    """Same external IO as build(), trivial body — for launch-overhead
    calibration."""
    HALF = H // 2
    SH_ROWS = HALF + 2
    NPX = HALF * W
    nc = bacc.Bacc("TRN2", target_bir_lowering=False, debug=False,
                   num_devices=N_CORES)
    dram = lambda n, s, d, kind: nc.dram_tensor(n, s, d, kind=kind).ap()
    x_d = dram("x", [C, SH_ROWS * WP], bf16, "ExternalInput")
    dram("wt1", [128, 576], bf16, "ExternalInput")
    dram("wt2", [64, 576], bf16, "ExternalInput")
    dram("wdw", [128, 45], f32, "ExternalInput")
    dram("pjt1", [128, 192], bf16, "ExternalInput")
    dram("pjt2", [64, 192], bf16, "ExternalInput")
    dram("tempb", [48, 4], f32, "ExternalInput")
    y_d = dram("y", [C, NPX], bf16, "ExternalOutput")
    with tile.TileContext(nc) as tc, ExitStack() as ctx:
        sb = ctx.enter_context(tc.tile_pool(name="sb", bufs=1))
        t = sb.tile([128, 200], bf16)
        nc.sync.dma_start(t[:, 0:169], x_d[0:128, 0:169])
        t2 = sb.tile([128, 200], bf16)
        nc.vector.tensor_copy(t2[:, 0:169], t[:, 0:169])
        nc.sync.dma_start(y_d[0:128, 0:169], t2[:, 0:169])
    nc.compile()
    return nc


def get_empty_runner(H=256):
    key = (H,)
    if key not in _EMPTY:
        nc = build_empty(H=H)
        _EMPTY[key] = (nc, _make_runner(nc))
    return _EMPTY[key][1]
